# revision 53
# baseline (speedup 1.0000x reference)
"""Multi-head attention (B=2, S=2048, D=1024, H=16, d_k=64) on 8 TRN2 NeuronCores.

Sharding: batch x head-groups. Core c handles batch b = c // 4 and heads
[4*(c%4), 4*(c%4)+4), i.e. a 256-wide slice of the model dim. Host sums the
4 partial y's per batch and adds bo. x inputs are host-transposed to [D, S]
so all x loads are plain contiguous DMAs (no DMA-transpose cost).

Per-core kernel. Two engines bound the runtime at ~142us busy each: the PE
(matmul cost = moving/free size per instruction) and ScalarE (softmax exp,
the only engine that can run activations). Everything is organized to keep
both saturated:
  - scores S^T = K Q^T per head-pair: two (64x128)-stationary matmuls per
    kt tile, free dim 512 (f32r, full rate),
  - attention out in [query, d_k] orientation: stationary = exp-score tile
    pt [128 keys, 128 queries], moving = ones-AUGMENTED V [128 keys, 65]
    (64 v-columns + a ones column), accumulated over the 16 key tiles in
    PSUM (one accumulation group per bank: start only on the first write,
    stop on the last - PSUM zero regions are 2KB). Free size is 65 instead
    of 512, and column 64 accumulates the softmax denominator for free,
  - normalization is partition-aligned: DVE reciprocal of the denominator
    column + tensor_scalar multiply per [128, 64] block -> attn [q, dk]
    bf16 in SBUF,
  - a PE transpose (identity matmul, 128 rows each) flips attn back to
    [dk, token] for the O-projection; 2 heads per transpose,
  - phase 1 chunk-interleaves the K projection (bias per 128-token chunk)
    with qb0's score+exp chain so the exp stream starts as soon as K/Q0
    land (~16us, DMA-bandwidth-bound) and never gaps; V projections are
    split between phase 1 (kt 0-7) and the qb0 loop (kt 8-15, emitted
    deterministically >= 2 kt ahead of their AV consumers),
  - Q(1..3) projections, next-qb scores, transposes and O-projection
    chunks flow through two ordered work queues (exp-feeding items vs
    PE-tail items) popped alternately as filler BEFORE the exp-gated AV
    matmuls (PE queue is in-order, so filler must precede the stall),
  - weight DMAs ride the ACT queue, x loads the SP queue, y stores the SP
    queue per 512-wide half; transpose + O-proj + late-projection PSUM
    tiles share one 2-deep tag ring so consecutive chunks do not
    serialize on the previous chunk's drain,
  - every hp1 block runs qt-major: each query tile accumulates over all
    16 kt, then its normalize -> transpose -> O-projection chain fires at
    once, so O-proj work lands inside the exp-covered span and the kernel
    tail pipelines per token tile; in the exp-free tail, copies and half
    the normalize multiplies shift to the idle ScalarE,
  - startup-critical loads (wk, xk0, wq, xq0) ride the SP queue in
    dependency order - the ACT queue is blocked ~1.3us at t=0 by the
    activation table load.

PSUM budget (8 banks): sct ring 4 | avq 2 (one bank per head) | shared
transpose/O-proj/projection ring 2 (phase 1: 2-bank kps ring + 2-bank
projection ring instead). Matmuls: f32r for scores (full rate at free dim
>= 256), bf16 elsewhere; f32 accumulation throughout.
"""

import numpy as np

B, S, D = 2, 2048, 1024
H, DK = 16, 64
NCORES = 8
DS = 256            # model-dim slice per core (4 heads x 64)
P = 128
DKA = DK + 1        # v columns + softmax-denominator ones column

_cache = {}


def _build(repeat=1):
    import concourse.mybir as mybir
    import concourse.tile as tile
    from concourse import bacc

    f32 = mybir.dt.float32
    f32r = mybir.dt.float32r
    bf16 = mybir.dt.bfloat16
    Exp = mybir.ActivationFunctionType.Exp
    Copy = mybir.ActivationFunctionType.Copy
    add = mybir.AluOpType.add
    mult = mybir.AluOpType.mult

    nc = bacc.Bacc("TRN2", target_bir_lowering=False, debug=False,
                   num_devices=NCORES)

    xq_d = nc.dram_tensor("xq", [D, S], bf16, kind="ExternalInput")
    xk_d = nc.dram_tensor("xk", [D, S], bf16, kind="ExternalInput")
    xv_d = nc.dram_tensor("xv", [D, S], bf16, kind="ExternalInput")
    wqT_d = nc.dram_tensor("wqT", [D, DS], bf16, kind="ExternalInput")
    wkT_d = nc.dram_tensor("wkT", [D, DS], bf16, kind="ExternalInput")
    wvT_d = nc.dram_tensor("wvT", [D, DS], bf16, kind="ExternalInput")
    woT_d = nc.dram_tensor("woT", [DS, D], bf16, kind="ExternalInput")
    bq_d = nc.dram_tensor("bq", [2, P, 1], f32, kind="ExternalInput")
    bk_d = nc.dram_tensor("bk", [2, P, 1], f32, kind="ExternalInput")
    bvr_d = nc.dram_tensor("bvr", [P, DS], bf16, kind="ExternalInput")
    id_d = nc.dram_tensor("ident", [P, P], bf16, kind="ExternalInput")
    y_d = nc.dram_tensor("y", [S, D], f32, kind="ExternalOutput")

    with tile.TileContext(nc) as tc:
        with (
            tc.tile_pool(name="persist", bufs=1) as pp,
            tc.tile_pool(name="xT", bufs=3) as xtp,
            tc.tile_pool(name="pt", bufs=52) as ptp,
            tc.tile_pool(name="attn", bufs=4) as asp,
            tc.tile_pool(name="small", bufs=2) as smp,
            tc.tile_pool(name="ysb", bufs=2) as yp,
        ):
            # ---- constants / weights ----
            wq_bf = pp.tile([P, 8, DS], bf16)
            wk_bf = pp.tile([P, 8, DS], bf16)
            wv_bf = pp.tile([P, 8, DS], bf16)
            wo_bf = pp.tile([P, 2, D], bf16)
            bq_sb = pp.tile([P, 2, 1], f32)
            bk_sb = pp.tile([P, 2, 1], f32)
            bv_sb = pp.tile([P, DS], bf16)
            id_sb = pp.tile([P, P], bf16)

            # ---- persistent activations ----
            QT = pp.tile([P, 2, S], f32r)      # [dk-in-pair, head-pair, token]
            KT = pp.tile([P, 2, S], f32r)
            V = pp.tile([P, 16, 4, DKA], bf16)  # [key-in-tile, kt, head, dk+1]
            attnT = pp.tile([P, 2, S], bf16)   # [dk-in-pair, head-pair, token]
            # softmax-denominator ones column, preset once
            nc.vector.memset(V[:, :, :, DK:DKA], 1.0)

            for _rep in range(repeat):
                sc_ctx = tc.tile_pool(name="sc_ps", bufs=2, space="PSUM")
                scp = sc_ctx.__enter__()
                tr_ctx = tc.tile_pool(name="tr_ps", bufs=2, space="PSUM")
                trp = tr_ctx.__enter__()

                # startup-critical loads ride the SP queue in dependency
                # order (the ACT queue is blocked ~1.3us by the activation
                # table load): wk, xk0, wq, xq0
                nc.sync.dma_start(
                    wk_bf[:], wkT_d.ap().rearrange("(c p) d -> p c d", p=P))

                warm = pp.tile([P, 128], bf16, name="warm", tag="warm") \
                    if _rep == 0 else warm
                if _rep == 0:
                    nc.vector.memset(warm[:], 0.0)
                # ~5us of dependency-free matmuls: keeps the PE busy (and
                # its clock-gate warm) through the startup DMA fill, so the
                # first projection matmuls run at full clock
                for _w in range(48):
                    wps = trp.tile([P, 512], f32, tag="pj", name="wps")
                    nc.tensor.matmul(wps[:, 0:128], warm[:], warm[:],
                                     start=True, stop=True)

                pts = {}

                def emit_scores(qb, kt, hps=(0, 1)):
                    qs = slice(qb * 512, (qb + 1) * 512)
                    for hp in hps:
                        sct = scp.tile([P, 2, 512], f32, tag="sct")
                        for hh in range(2):
                            hb = 64 * hh
                            nc.tensor.matmul(
                                sct[:, hh, :],
                                KT[hb:hb + 64, hp, kt * P:(kt + 1) * P],
                                QT[hb:hb + 64, hp, qs],
                                start=True, stop=True)
                        pt = ptp.tile([P, 2, 512], bf16, tag="pt")
                        nc.scalar.activation(pt[:], sct[:], Exp, scale=0.125)
                        pts[(qb, kt, hp)] = pt

                def load_xT(x_d, t4, split=1):
                    # x comes host-transposed [D, S]: plain contiguous DMA
                    xT = xtp.tile([P, 8, 512], bf16, tag="xT")
                    xv = x_d.ap().rearrange("(c p) (f s t) -> p c f s t",
                                            p=P, f=4, s=split)
                    w = 512 // split
                    for s in range(split):
                        nc.sync.dma_start(xT[:, :, s * w:(s + 1) * w],
                                          xv[:, :, t4, s, :])
                    return xT

                def emit_proj(kind, t4, xT, hp, fine=False, pool=None,
                              css=None, keep=None):
                    w = wk_bf if kind == "k" else wq_bf
                    bias = bk_sb if kind == "k" else bq_sb
                    out = KT if kind == "k" else QT
                    # fine=True: 128-token chunks so the first matmuls start
                    # after the first split-load lands, not after all four
                    chunks = [slice(128 * i, 128 * (i + 1))
                              for i in (range(4) if css is None else css)] \
                        if fine else [slice(0, 512)]
                    if keep is not None:
                        ps = keep
                    elif pool is None:
                        ps = trp.tile([P, 512], f32, tag="pj", name="ps")
                    else:
                        ps = pool.tile([P, 512], f32, tag="tpy", name="ps")
                    ob = t4 * 512
                    for cs in chunks:
                        for ch in range(8):
                            nc.tensor.matmul(
                                ps[:, cs], w[:, ch, hp * P:(hp + 1) * P],
                                xT[:, ch, cs],
                                start=(ch == 0), stop=(ch == 7))
                        if fine:
                            nc.vector.tensor_scalar(
                                out[:, hp, ob + cs.start:ob + cs.stop],
                                ps[:, cs], bias[:, hp, :], None, op0=add)
                    if not fine:
                        nc.vector.tensor_scalar(
                            out[:, hp, ob:ob + 512], ps[:],
                            bias[:, hp, :], None, op0=add)
                    return ps

                def emit_v_half(xT, t4, half, pool=None):
                    if pool is None:
                        pv = trp.tile([P, 512], f32, tag="pj", name="pv")
                    else:
                        pv = pool.tile([P, 512], f32, tag="tpy", name="pv")
                    pvv = pv[:].rearrange("p (t d) -> p t d", t=2)
                    for j in range(2):
                        ti = 2 * half + j
                        for ch in range(8):
                            nc.tensor.matmul(
                                pvv[:, j, :],
                                xT[:, ch, ti * P:(ti + 1) * P],
                                wv_bf[:, ch, :],
                                start=(ch == 0), stop=(ch == 7))
                    for j in range(2):
                        tb = 4 * t4 + 2 * half + j
                        nc.vector.tensor_add(
                            V[:, tb, :, 0:DK],
                            pvv[:, j, :].rearrange("p (h d) -> p h d", h=4),
                            bv_sb[:].rearrange("p (h d) -> p h d", h=4))

                # ---- phase 1: K/V/Q0 projections fused with qb0 scores;
                # K is chunk-interleaved so each score tile emits right
                # after its 128-token K chunk and the exp stream never gaps
                xq_tiles = {}
                xv_tiles = {}
                for t4 in range(4):
                    xTk = load_xT(xk_d, t4, split=2 if t4 == 0 else 1)
                    if t4 == 0:
                        nc.sync.dma_start(
                            wq_bf[:],
                            wqT_d.ap().rearrange("(c p) d -> p c d", p=P))
                        xq_tiles[0] = load_xT(xq_d, 0, split=2)
                        nc.scalar.dma_start(bk_sb[:, 0, :], bk_d.ap()[0])
                        nc.scalar.dma_start(bk_sb[:, 1, :], bk_d.ap()[1])
                        nc.scalar.dma_start(bq_sb[:, 0, :], bq_d.ap()[0])
                        nc.scalar.dma_start(bq_sb[:, 1, :], bq_d.ap()[1])
                        # interleave K/Q chunk halves so projection
                        # compute overlaps the trailing x/w transfers
                        ps_k = {}
                        ps_q = {}
                        for half in range(2):
                            cs2 = [2 * half, 2 * half + 1]
                            for hp in range(2):
                                ps_k[hp] = emit_proj(
                                    "k", 0, xTk, hp, fine=True, css=cs2,
                                    keep=ps_k.get(hp))
                            for hp in range(2):
                                ps_q[hp] = emit_proj(
                                    "q", 0, xq_tiles[0], hp, fine=True,
                                    css=cs2, keep=ps_q.get(hp))
                        # deferred constant loads, off the startup DMA path
                        nc.scalar.dma_start(
                            wv_bf[:],
                            wvT_d.ap().rearrange("(c p) d -> p c d", p=P))
                        nc.scalar.dma_start(bv_sb[:], bvr_d.ap())
                        nc.scalar.dma_start(id_sb[:], id_d.ap())
                        nc.scalar.dma_start(
                            wo_bf[:],
                            woT_d.ap().rearrange("(c p) d -> p c d", p=P))
                        emit_scores(0, 0)
                        emit_scores(0, 1)
                        emit_scores(0, 2)
                        emit_scores(0, 3)
                        xv_tiles[0] = load_xT(xv_d, 0)
                        continue
                    if t4 == 1:
                        xv_tiles[1] = load_xT(xv_d, 1)
                    if t4 == 2:
                        xq_tiles[1] = load_xT(xq_d, 1)
                    kps = [trp.tile([P, 512], f32, tag="kps", name=f"kps{_h}")
                           for _h in range(2)]
                    for i in range(4):
                        cs = slice(128 * i, 128 * (i + 1))
                        for hp in range(2):
                            for ch in range(8):
                                nc.tensor.matmul(
                                    kps[hp][:, cs],
                                    wk_bf[:, ch, hp * P:(hp + 1) * P],
                                    xTk[:, ch, cs],
                                    start=(ch == 0), stop=(ch == 7))
                            nc.vector.tensor_scalar(
                                KT[:, hp, t4 * 512 + cs.start:
                                   t4 * 512 + cs.stop],
                                kps[hp][:, cs], bk_sb[:, hp, :],
                                None, op0=add)
                        emit_scores(0, 4 * t4 + i)
                        # spread V/Q(1) extras evenly (one per ~2 chunks)
                        # so no single burst outruns the 2-tile sct ring;
                        # t4 2/3's V halves move to the qb0 work queue
                        # where the exp stream, not the PE, binds
                        if i in (1, 3):
                            ex = [("V", 0, 0), ("V", 0, 1),
                                  ("V", 1, 0), ("V", 1, 1),
                                  ("Q", 1, 0), ("Q", 1, 1)][
                                      2 * (t4 - 1) + (i - 1) // 2]
                            if ex[0] == "V":
                                emit_v_half(xv_tiles[ex[1]], ex[1], ex[2])
                            else:
                                emit_proj("q", 1, xq_tiles[1], ex[2])

                # phase 1 projection PSUM ring -> AV + transpose/O-proj rings
                tr_ctx.__exit__(None, None, None)
                av_ctx = tc.tile_pool(name="av_ps", bufs=2, space="PSUM")
                avp = av_ctx.__enter__()
                ty_ctx = tc.tile_pool(name="ty_ps", bufs=2, space="PSUM")
                typ = ty_ctx.__enter__()

                # ---- attention (hp-outer) + work-queue filler ----
                # two queues popped alternately: score/projection items feed
                # the Scalar engine, transpose/O-proj items feed the PE tail
                work = []
                work_ty = []
                work_s3 = []
                tog = [0]
                attn_sbs = {}
                y_sbs = {}

                def emit_T(qb, hp, qts=range(4)):
                    # PE transposes attn [q, dk] -> attnT [dk, q]; 2 heads
                    # stack per instruction via the [q, (hh dk)] input view
                    tpt = typ.tile([P, 4, P], bf16, tag="tpy", name="tpt")
                    a_sb = attn_sbs[(qb, hp)]
                    for qt in qts:
                        nc.tensor.transpose(tpt[:, qt, :],
                                            a_sb[:, qt, :, :], id_sb[:])
                        dst = attnT[:, hp, (4 * qb + qt) * P:
                                    (4 * qb + qt + 1) * P]
                        if tail[0] and qt % 2 == 0:
                            nc.scalar.copy(dst, tpt[:, qt, :])
                        else:
                            nc.vector.tensor_copy(dst, tpt[:, qt, :])

                tail = [False]

                def emit_y_half(tt, nb):
                    if tt not in y_sbs:
                        y_sbs[tt] = yp.tile([P, D], f32, name="y_sb", tag="y")
                    y_sb = y_sbs[tt]
                    py = typ.tile([P, 512], f32, tag="tpy", name="py")
                    for hpc in range(2):
                        nc.tensor.matmul(
                            py[:],
                            attnT[:, hpc, tt * P:(tt + 1) * P],
                            wo_bf[:, hpc, nb * 512:(nb + 1) * 512],
                            start=(hpc == 0), stop=(hpc == 1))
                    if tail[0] and (tt + nb) % 2 == 0:
                        nc.scalar.copy(y_sb[:, nb * 512:(nb + 1) * 512],
                                       py[:])
                    else:
                        nc.vector.tensor_copy(
                            y_sb[:, nb * 512:(nb + 1) * 512], py[:])
                    nc.sync.dma_start(
                        y_d.ap()[tt * P:(tt + 1) * P,
                                 nb * 512:(nb + 1) * 512],
                        y_sb[:, nb * 512:(nb + 1) * 512])
                    if nb == 1:
                        del y_sbs[tt]

                def emit_work(n):
                    for _ in range(n):
                        tog[0] ^= 1
                        if work and (tog[0] or not work_ty):
                            kind, *a = work.pop(0)
                        elif work_ty:
                            kind, *a = work_ty.pop(0)
                        else:
                            return
                        if kind == "S":
                            emit_scores(*a)
                        elif kind == "XQ":
                            xq_tiles[a[0]] = load_xT(xq_d, a[0])
                        elif kind == "XV":
                            xv_tiles[a[0]] = load_xT(xv_d, a[0])
                        elif kind == "V":
                            emit_v_half(xv_tiles[a[0]], a[0], a[1],
                                        pool=typ)
                        elif kind == "Q":
                            emit_proj("q", a[0], xq_tiles[a[0]], a[1],
                                      pool=typ)
                        elif kind == "T":
                            emit_T(*a)
                        else:
                            emit_y_half(*a)

                for qb in range(4):
                    if qb == 3:
                        tail[0] = True
                    if qb < 3:
                        # next-qb Q projection + scores feed the queue;
                        # Q/XQ items spread between S items so the exp
                        # stream never pauses for a projection burst
                        ns = [("S", qb + 1, kt) for kt in range(16)]
                        if qb == 0:
                            ns2 = [("S", 2, kt) for kt in range(6)]
                            work.extend([("XQ", 2)])
                            work.extend(ns[0:4] + [("Q", 2, 0)] + ns[4:8]
                                        + [("XQ", 3)] + ns[8:12]
                                        + [("Q", 2, 1)] + ns[12:16] + ns2)
                        elif qb == 1:
                            ns = [("S", 2, kt) for kt in range(6, 16)]
                            work.extend(ns[0:4] + [("Q", 3, 0)] + ns[4:8]
                                        + [("Q", 3, 1)] + ns[8:]
                                        + [("S", 3, kt) for kt in range(4)])
                        else:
                            work.extend([("S", 3, kt) for kt in range(4, 16)])

                    for hp in range(2):
                        avq = [avp.tile([P, 4, DKA], f32, tag="avq",
                                        name=f"avq{_h}")
                               for _h in range(2)]
                        if hp == 1:
                            # late blocks run qt-major: each query tile
                            # accumulates over all 16 kt, then its norm ->
                            # transpose -> O-projection chain fires at once.
                            # This executes T/Y work inside the exp-covered
                            # span (qb2) and pipelines the kernel tail per
                            # token tile (qb3). Drain the T/Y queue first so
                            # T(qb,0) always precedes this block's Ys.
                            while work_ty:
                                knd, *a = work_ty.pop(0)
                                if knd == "T":
                                    emit_T(*a)
                                else:
                                    emit_y_half(*a)
                            a_sb = asp.tile([P, 4, 2, DK], bf16, tag="attn")
                            attn_sbs[(qb, hp)] = a_sb
                            for qt in range(4):
                                for kt in range(16):
                                    if kt % 4 == 0:
                                        emit_work(1)
                                    for hh in range(2):
                                        nc.tensor.matmul(
                                            avq[hh][:, qt, :],
                                            pts[(qb, kt, hp)][
                                                :, hh, qt * P:(qt + 1) * P],
                                            V[:, kt, 2 * hp + hh, :],
                                            start=(kt == 0), stop=(kt == 15))
                                for hh in range(2):
                                    rec2 = smp.tile([P, 1], f32, tag="rec2",
                                                    name="rec2")
                                    nc.vector.reciprocal(
                                        rec2[:], avq[hh][:, qt, DK:DKA])
                                    if tail[0] and hh == 1:
                                        nc.scalar.activation(
                                            a_sb[:, qt, hh, :],
                                            avq[hh][:, qt, 0:DK], Copy,
                                            scale=rec2[:])
                                    else:
                                        nc.vector.tensor_scalar(
                                            a_sb[:, qt, hh, :],
                                            avq[hh][:, qt, 0:DK],
                                            rec2[:], None, op0=mult)
                                emit_T(qb, hp, qts=(qt,))
                                if hp == 1:
                                    emit_y_half(4 * qb + qt, 0)
                                    emit_y_half(4 * qb + qt, 1)
                            for kt in range(16):
                                del pts[(qb, kt, hp)]
                            continue
                        for kt in range(16):
                            # PE queue is in-order: pop independent filler
                            # BEFORE the exp-gated AV matmuls so it runs
                            # during the wait. The t4=2/3 V projections are
                            # emitted here deterministically, always >= 2 kt
                            # ahead of the AV matmuls that read them.
                            if qb == 0 and hp == 0 and kt in (1, 3):
                                emit_work(1)
                            if qb == 0 and hp == 0:
                                if kt == 0:
                                    xv_tiles[2] = load_xT(xv_d, 2)
                                elif kt == 2:
                                    xv_tiles[3] = load_xT(xv_d, 3)
                                elif kt in (4, 6, 8, 10):
                                    t4v = 2 + (kt - 4) // 4
                                    emit_v_half(xv_tiles[t4v], t4v,
                                                (kt // 2) % 2, pool=typ)
                            if kt % 2 == 0:
                                emit_work(1)
                            if (hp * 16 + kt) % 4 == 0:
                                emit_work(1)
                            ptm = pts[(qb, kt, hp)]
                            for hh in range(2):
                                h = 2 * hp + hh
                                for qt in range(4):
                                    # one PSUM accumulation group per bank:
                                    # start marks the whole 2KB zero region,
                                    # each qt's first write then zero-fills
                                    nc.tensor.matmul(
                                        avq[hh][:, qt, :],
                                        ptm[:, hh, qt * P:(qt + 1) * P],
                                        V[:, kt, h, :],
                                        start=(kt == 0 and qt == 0),
                                        stop=(kt == 15 and qt == 3))
                            del pts[(qb, kt, hp)]
                        # boundary filler between the final AV and the DVE
                        # normalization chain
                        emit_work(2)
                        # partition-aligned normalization: reciprocal of the
                        # accumulated denominator column, then one
                        # tensor_scalar multiply per [128, 64] block
                        a_sb = asp.tile([P, 4, 2, DK], bf16, tag="attn")
                        attn_sbs[(qb, hp)] = a_sb
                        for hh in range(2):
                            rec = smp.tile([P, 4, 1], f32, tag="rec")
                            nc.vector.reciprocal(rec[:],
                                                 avq[hh][:, :, DK:DKA])
                            for qt in range(4):
                                # in the exp-free tail, hh1's normalize
                                # multiplies run on the idle Scalar engine
                                # (Copy activation with per-partition scale)
                                if tail[0] and hh == 1:
                                    nc.scalar.activation(
                                        a_sb[:, qt, hh, :],
                                        avq[hh][:, qt, 0:DK], Copy,
                                        scale=rec[:, qt, :])
                                else:
                                    nc.vector.tensor_scalar(
                                        a_sb[:, qt, hh, :],
                                        avq[hh][:, qt, 0:DK],
                                        rec[:, qt, :], None, op0=mult)
                        work_ty.append(("T", qb, hp))
                        if hp == 1 and qb < 3:
                                for tt in range(4 * qb, 4 * qb + 4):
                                    for nb in range(2):
                                        work_ty.append(("Y", tt, nb))
                emit_work(len(work) + len(work_ty))

                for ctx in (ty_ctx, av_ctx, sc_ctx):
                    ctx.__exit__(None, None, None)

    nc.compile()
    return nc


def _shard(query, key, value, Wq, bq, Wk, bk, Wv, bv, Wo, bo):
    import ml_dtypes
    f = np.float32
    bf = ml_dtypes.bfloat16
    q = np.asarray(query, dtype=f).astype(bf)
    k = np.asarray(key, dtype=f).astype(bf)
    v = np.asarray(value, dtype=f).astype(bf)
    qT = [np.ascontiguousarray(q[b].T) for b in range(B)]
    kT = [np.ascontiguousarray(k[b].T) for b in range(B)]
    vT = [np.ascontiguousarray(v[b].T) for b in range(B)]
    ident = np.eye(P, dtype=bf)
    in_maps = []
    for c in range(NCORES):
        b, hg = c // 4, c % 4
        ds = DS * hg
        bv_r = np.broadcast_to(
            np.asarray(bv, f)[ds:ds + DS].astype(bf)[None, :], (P, DS))
        in_maps.append({
            "xq": qT[b],
            "xk": kT[b],
            "xv": vT[b],
            "wqT": np.ascontiguousarray(np.asarray(Wq, f)[ds:ds + DS, :].T.astype(bf)),
            "wkT": np.ascontiguousarray(np.asarray(Wk, f)[ds:ds + DS, :].T.astype(bf)),
            "wvT": np.ascontiguousarray(np.asarray(Wv, f)[ds:ds + DS, :].T.astype(bf)),
            "woT": np.ascontiguousarray(np.asarray(Wo, f)[:, ds:ds + DS].T.astype(bf)),
            "bq": np.asarray(bq, f)[ds:ds + DS].reshape(2, P, 1),
            "bk": np.asarray(bk, f)[ds:ds + DS].reshape(2, P, 1),
            "bvr": np.ascontiguousarray(bv_r),
            "ident": ident,
        })
    return in_maps


def _unshard(results, bo):
    y = np.zeros((B, S, D), dtype=np.float64)
    for c in range(NCORES):
        y[c // 4] += results[c]["y"].astype(np.float64)
    y += np.asarray(bo, np.float64)
    return y.astype(np.float32)


def kernel(query, key, value, Wq, bq, Wk, bk, Wv, bv, Wo, bo):
    from concourse.bass_utils import run_bass_kernel_spmd

    if "nc" not in _cache:
        _cache["nc"] = _build()
    nc = _cache["nc"]
    in_maps = _shard(query, key, value, Wq, bq, Wk, bk, Wv, bv, Wo, bo)
    res = run_bass_kernel_spmd(nc, in_maps, core_ids=list(range(NCORES)))
    return _unshard(res.results, bo)


# revision 54
# speedup vs baseline: 1.0015x; 1.0015x over previous
"""Multi-head attention (B=2, S=2048, D=1024, H=16, d_k=64) on 8 TRN2 NeuronCores.

Sharding: batch x head-groups. Core c handles batch b = c // 4 and heads
[4*(c%4), 4*(c%4)+4), i.e. a 256-wide slice of the model dim. Host sums the
4 partial y's per batch and adds bo. x inputs are host-transposed to [D, S]
so all x loads are plain contiguous DMAs (no DMA-transpose cost).

Per-core kernel. Two engines bound the runtime at ~142us busy each: the PE
(matmul cost = moving/free size per instruction) and ScalarE (softmax exp,
the only engine that can run activations). Everything is organized to keep
both saturated:
  - scores S^T = K Q^T per head-pair: two (64x128)-stationary matmuls per
    kt tile, free dim 512 (f32r, full rate),
  - attention out in [query, d_k] orientation: stationary = exp-score tile
    pt [128 keys, 128 queries], moving = ones-AUGMENTED V [128 keys, 65]
    (64 v-columns + a ones column), accumulated over the 16 key tiles in
    PSUM (one accumulation group per bank: start only on the first write,
    stop on the last - PSUM zero regions are 2KB). Free size is 65 instead
    of 512, and column 64 accumulates the softmax denominator for free,
  - normalization is partition-aligned: DVE reciprocal of the denominator
    column + tensor_scalar multiply per [128, 64] block -> attn [q, dk]
    bf16 in SBUF,
  - a PE transpose (identity matmul, 128 rows each) flips attn back to
    [dk, token] for the O-projection; 2 heads per transpose,
  - phase 1 chunk-interleaves the K projection (bias per 128-token chunk)
    with qb0's score+exp chain so the exp stream starts as soon as K/Q0
    land (~16us, DMA-bandwidth-bound) and never gaps; V projections are
    split between phase 1 (kt 0-7) and the qb0 loop (kt 8-15, emitted
    deterministically >= 2 kt ahead of their AV consumers),
  - Q(1..3) projections, next-qb scores, transposes and O-projection
    chunks flow through two ordered work queues (exp-feeding items vs
    PE-tail items) popped alternately as filler BEFORE the exp-gated AV
    matmuls (PE queue is in-order, so filler must precede the stall),
  - weight DMAs ride the ACT queue, x loads the SP queue, y stores the SP
    queue per 512-wide half; transpose + O-proj + late-projection PSUM
    tiles share one 2-deep tag ring so consecutive chunks do not
    serialize on the previous chunk's drain,
  - every hp1 block runs qt-major: each query tile accumulates over all
    16 kt, then its normalize -> transpose -> O-projection chain fires at
    once, so O-proj work lands inside the exp-covered span and the kernel
    tail pipelines per token tile; in the exp-free tail, copies and half
    the normalize multiplies shift to the idle ScalarE,
  - startup-critical loads (wk, xk0, wq, xq0) ride the SP queue in
    dependency order - the ACT queue is blocked ~1.3us at t=0 by the
    activation table load.

PSUM budget (8 banks): sct ring 4 | avq 2 (one bank per head) | shared
transpose/O-proj/projection ring 2 (phase 1: 2-bank kps ring + 2-bank
projection ring instead). Matmuls: f32r for scores (full rate at free dim
>= 256), bf16 elsewhere; f32 accumulation throughout.
"""

import numpy as np

B, S, D = 2, 2048, 1024
H, DK = 16, 64
NCORES = 8
DS = 256            # model-dim slice per core (4 heads x 64)
P = 128
DKA = DK + 1        # v columns + softmax-denominator ones column

_cache = {}


def _build(repeat=1):
    import concourse.mybir as mybir
    import concourse.tile as tile
    from concourse import bacc

    f32 = mybir.dt.float32
    f32r = mybir.dt.float32r
    bf16 = mybir.dt.bfloat16
    Exp = mybir.ActivationFunctionType.Exp
    Copy = mybir.ActivationFunctionType.Copy
    add = mybir.AluOpType.add
    mult = mybir.AluOpType.mult

    nc = bacc.Bacc("TRN2", target_bir_lowering=False, debug=False,
                   num_devices=NCORES)

    xq_d = nc.dram_tensor("xq", [D, S], bf16, kind="ExternalInput")
    xk_d = nc.dram_tensor("xk", [D, S], bf16, kind="ExternalInput")
    xv_d = nc.dram_tensor("xv", [D, S], bf16, kind="ExternalInput")
    wqT_d = nc.dram_tensor("wqT", [D, DS], bf16, kind="ExternalInput")
    wkT_d = nc.dram_tensor("wkT", [D, DS], bf16, kind="ExternalInput")
    wvT_d = nc.dram_tensor("wvT", [D, DS], bf16, kind="ExternalInput")
    woT_d = nc.dram_tensor("woT", [DS, D], bf16, kind="ExternalInput")
    bq_d = nc.dram_tensor("bq", [2, P, 1], f32, kind="ExternalInput")
    bk_d = nc.dram_tensor("bk", [2, P, 1], f32, kind="ExternalInput")
    bvr_d = nc.dram_tensor("bvr", [P, DS], bf16, kind="ExternalInput")
    id_d = nc.dram_tensor("ident", [P, P], bf16, kind="ExternalInput")
    y_d = nc.dram_tensor("y", [S, D], f32, kind="ExternalOutput")

    with tile.TileContext(nc) as tc:
        with (
            tc.tile_pool(name="persist", bufs=1) as pp,
            tc.tile_pool(name="xT", bufs=3) as xtp,
            tc.tile_pool(name="pt", bufs=52) as ptp,
            tc.tile_pool(name="attn", bufs=4) as asp,
            tc.tile_pool(name="small", bufs=2) as smp,
            tc.tile_pool(name="ysb", bufs=2) as yp,
        ):
            # ---- constants / weights ----
            wq_bf = pp.tile([P, 8, DS], bf16)
            wk_bf = pp.tile([P, 8, DS], bf16)
            wv_bf = pp.tile([P, 8, DS], bf16)
            wo_bf = pp.tile([P, 2, D], bf16)
            bq_sb = pp.tile([P, 2, 1], f32)
            bk_sb = pp.tile([P, 2, 1], f32)
            bv_sb = pp.tile([P, DS], bf16)
            id_sb = pp.tile([P, P], bf16)

            # ---- persistent activations ----
            QT = pp.tile([P, 2, S], f32r)      # [dk-in-pair, head-pair, token]
            KT = pp.tile([P, 2, S], f32r)
            V = pp.tile([P, 16, 4, DKA], bf16)  # [key-in-tile, kt, head, dk+1]
            attnT = pp.tile([P, 2, S], bf16)   # [dk-in-pair, head-pair, token]
            # softmax-denominator ones column, preset once
            nc.vector.memset(V[:, :, :, DK:DKA], 1.0)

            for _rep in range(repeat):
                sc_ctx = tc.tile_pool(name="sc_ps", bufs=2, space="PSUM")
                scp = sc_ctx.__enter__()
                tr_ctx = tc.tile_pool(name="tr_ps", bufs=2, space="PSUM")
                trp = tr_ctx.__enter__()

                # startup-critical loads ride the SP queue in dependency
                # order (the ACT queue is blocked ~1.3us by the activation
                # table load): wk, xk0, wq, xq0
                nc.sync.dma_start(
                    wk_bf[:], wkT_d.ap().rearrange("(c p) d -> p c d", p=P))

                warm = pp.tile([P, 128], bf16, name="warm", tag="warm") \
                    if _rep == 0 else warm
                if _rep == 0:
                    nc.vector.memset(warm[:], 0.0)
                # ~5us of dependency-free matmuls: keeps the PE busy (and
                # its clock-gate warm) through the startup DMA fill, so the
                # first projection matmuls run at full clock
                for _w in range(48):
                    wps = trp.tile([P, 512], f32, tag="pj", name="wps")
                    nc.tensor.matmul(wps[:, 0:128], warm[:], warm[:],
                                     start=True, stop=True)

                pts = {}

                def emit_scores(qb, kt, hps=(0, 1)):
                    qs = slice(qb * 512, (qb + 1) * 512)
                    for hp in hps:
                        sct = scp.tile([P, 2, 512], f32, tag="sct")
                        for hh in range(2):
                            hb = 64 * hh
                            nc.tensor.matmul(
                                sct[:, hh, :],
                                KT[hb:hb + 64, hp, kt * P:(kt + 1) * P],
                                QT[hb:hb + 64, hp, qs],
                                start=True, stop=True)
                        pt = ptp.tile([P, 2, 512], bf16, tag="pt")
                        nc.scalar.activation(pt[:], sct[:], Exp, scale=0.125)
                        pts[(qb, kt, hp)] = pt

                def load_xT(x_d, t4, split=1):
                    # x comes host-transposed [D, S]: plain contiguous DMA
                    xT = xtp.tile([P, 8, 512], bf16, tag="xT")
                    xv = x_d.ap().rearrange("(c p) (f s t) -> p c f s t",
                                            p=P, f=4, s=split)
                    w = 512 // split
                    for s in range(split):
                        nc.sync.dma_start(xT[:, :, s * w:(s + 1) * w],
                                          xv[:, :, t4, s, :])
                    return xT

                def emit_proj(kind, t4, xT, hp, fine=False, pool=None,
                              css=None, keep=None):
                    w = wk_bf if kind == "k" else wq_bf
                    bias = bk_sb if kind == "k" else bq_sb
                    out = KT if kind == "k" else QT
                    # fine=True: 128-token chunks so the first matmuls start
                    # after the first split-load lands, not after all four
                    chunks = [slice(128 * i, 128 * (i + 1))
                              for i in (range(4) if css is None else css)] \
                        if fine else [slice(0, 512)]
                    if keep is not None:
                        ps = keep
                    elif pool is None:
                        ps = trp.tile([P, 512], f32, tag="pj", name="ps")
                    else:
                        ps = pool.tile([P, 512], f32, tag="tpy", name="ps")
                    ob = t4 * 512
                    for cs in chunks:
                        for ch in range(8):
                            nc.tensor.matmul(
                                ps[:, cs], w[:, ch, hp * P:(hp + 1) * P],
                                xT[:, ch, cs],
                                start=(ch == 0), stop=(ch == 7))
                        if fine:
                            nc.vector.tensor_scalar(
                                out[:, hp, ob + cs.start:ob + cs.stop],
                                ps[:, cs], bias[:, hp, :], None, op0=add)
                    if not fine:
                        nc.vector.tensor_scalar(
                            out[:, hp, ob:ob + 512], ps[:],
                            bias[:, hp, :], None, op0=add)
                    return ps

                def emit_v_half(xT, t4, half, pool=None):
                    if pool is None:
                        pv = trp.tile([P, 512], f32, tag="pj", name="pv")
                    else:
                        pv = pool.tile([P, 512], f32, tag="tpy", name="pv")
                    pvv = pv[:].rearrange("p (t d) -> p t d", t=2)
                    for j in range(2):
                        ti = 2 * half + j
                        for ch in range(8):
                            nc.tensor.matmul(
                                pvv[:, j, :],
                                xT[:, ch, ti * P:(ti + 1) * P],
                                wv_bf[:, ch, :],
                                start=(ch == 0), stop=(ch == 7))
                    for j in range(2):
                        tb = 4 * t4 + 2 * half + j
                        nc.vector.tensor_add(
                            V[:, tb, :, 0:DK],
                            pvv[:, j, :].rearrange("p (h d) -> p h d", h=4),
                            bv_sb[:].rearrange("p (h d) -> p h d", h=4))

                # ---- phase 1: K/V/Q0 projections fused with qb0 scores;
                # K is chunk-interleaved so each score tile emits right
                # after its 128-token K chunk and the exp stream never gaps
                xq_tiles = {}
                xv_tiles = {}
                for t4 in range(4):
                    xTk = load_xT(xk_d, t4, split=2 if t4 == 0 else 1)
                    if t4 == 0:
                        nc.sync.dma_start(
                            wq_bf[:],
                            wqT_d.ap().rearrange("(c p) d -> p c d", p=P))
                        xq_tiles[0] = load_xT(xq_d, 0, split=2)
                        # tiny latency-critical loads ride the GPSIMD
                        # SWDGE queue: no HWDGE slot, no ACT-queue delay
                        nc.gpsimd.dma_start(bk_sb[:, 0, :], bk_d.ap()[0])
                        nc.gpsimd.dma_start(bk_sb[:, 1, :], bk_d.ap()[1])
                        nc.gpsimd.dma_start(bq_sb[:, 0, :], bq_d.ap()[0])
                        nc.gpsimd.dma_start(bq_sb[:, 1, :], bq_d.ap()[1])
                        # interleave K/Q chunk halves so projection
                        # compute overlaps the trailing x/w transfers
                        ps_k = {}
                        ps_q = {}
                        for half in range(2):
                            cs2 = [2 * half, 2 * half + 1]
                            for hp in range(2):
                                ps_k[hp] = emit_proj(
                                    "k", 0, xTk, hp, fine=True, css=cs2,
                                    keep=ps_k.get(hp))
                            for hp in range(2):
                                ps_q[hp] = emit_proj(
                                    "q", 0, xq_tiles[0], hp, fine=True,
                                    css=cs2, keep=ps_q.get(hp))
                        # deferred constant loads, off the startup DMA path
                        nc.scalar.dma_start(
                            wv_bf[:],
                            wvT_d.ap().rearrange("(c p) d -> p c d", p=P))
                        nc.gpsimd.dma_start(bv_sb[:], bvr_d.ap())
                        nc.gpsimd.dma_start(id_sb[:], id_d.ap())
                        nc.scalar.dma_start(
                            wo_bf[:],
                            woT_d.ap().rearrange("(c p) d -> p c d", p=P))
                        emit_scores(0, 0)
                        emit_scores(0, 1)
                        emit_scores(0, 2)
                        emit_scores(0, 3)
                        xv_tiles[0] = load_xT(xv_d, 0)
                        continue
                    if t4 == 1:
                        xv_tiles[1] = load_xT(xv_d, 1)
                    if t4 == 2:
                        xq_tiles[1] = load_xT(xq_d, 1)
                    kps = [trp.tile([P, 512], f32, tag="kps", name=f"kps{_h}")
                           for _h in range(2)]
                    for i in range(4):
                        cs = slice(128 * i, 128 * (i + 1))
                        for hp in range(2):
                            for ch in range(8):
                                nc.tensor.matmul(
                                    kps[hp][:, cs],
                                    wk_bf[:, ch, hp * P:(hp + 1) * P],
                                    xTk[:, ch, cs],
                                    start=(ch == 0), stop=(ch == 7))
                            nc.vector.tensor_scalar(
                                KT[:, hp, t4 * 512 + cs.start:
                                   t4 * 512 + cs.stop],
                                kps[hp][:, cs], bk_sb[:, hp, :],
                                None, op0=add)
                        emit_scores(0, 4 * t4 + i)
                        # spread V/Q(1) extras evenly (one per ~2 chunks)
                        # so no single burst outruns the 2-tile sct ring;
                        # t4 2/3's V halves move to the qb0 work queue
                        # where the exp stream, not the PE, binds
                        if i in (1, 3):
                            ex = [("V", 0, 0), ("V", 0, 1),
                                  ("V", 1, 0), ("V", 1, 1),
                                  ("Q", 1, 0), ("Q", 1, 1)][
                                      2 * (t4 - 1) + (i - 1) // 2]
                            if ex[0] == "V":
                                emit_v_half(xv_tiles[ex[1]], ex[1], ex[2])
                            else:
                                emit_proj("q", 1, xq_tiles[1], ex[2])

                # phase 1 projection PSUM ring -> AV + transpose/O-proj rings
                tr_ctx.__exit__(None, None, None)
                av_ctx = tc.tile_pool(name="av_ps", bufs=2, space="PSUM")
                avp = av_ctx.__enter__()
                ty_ctx = tc.tile_pool(name="ty_ps", bufs=2, space="PSUM")
                typ = ty_ctx.__enter__()

                # ---- attention (hp-outer) + work-queue filler ----
                # two queues popped alternately: score/projection items feed
                # the Scalar engine, transpose/O-proj items feed the PE tail
                work = []
                work_ty = []
                work_s3 = []
                tog = [0]
                attn_sbs = {}
                y_sbs = {}

                def emit_T(qb, hp, qts=range(4)):
                    # PE transposes attn [q, dk] -> attnT [dk, q]; 2 heads
                    # stack per instruction via the [q, (hh dk)] input view
                    tpt = typ.tile([P, 4, P], bf16, tag="tpy", name="tpt")
                    a_sb = attn_sbs[(qb, hp)]
                    for qt in qts:
                        nc.tensor.transpose(tpt[:, qt, :],
                                            a_sb[:, qt, :, :], id_sb[:])
                        dst = attnT[:, hp, (4 * qb + qt) * P:
                                    (4 * qb + qt + 1) * P]
                        if tail[0] and qt % 2 == 0:
                            nc.scalar.copy(dst, tpt[:, qt, :])
                        else:
                            nc.vector.tensor_copy(dst, tpt[:, qt, :])

                tail = [False]

                def emit_y_half(tt, nb):
                    if tt not in y_sbs:
                        y_sbs[tt] = yp.tile([P, D], f32, name="y_sb", tag="y")
                    y_sb = y_sbs[tt]
                    py = typ.tile([P, 512], f32, tag="tpy", name="py")
                    for hpc in range(2):
                        nc.tensor.matmul(
                            py[:],
                            attnT[:, hpc, tt * P:(tt + 1) * P],
                            wo_bf[:, hpc, nb * 512:(nb + 1) * 512],
                            start=(hpc == 0), stop=(hpc == 1))
                    if tail[0] and (tt + nb) % 2 == 0:
                        nc.scalar.copy(y_sb[:, nb * 512:(nb + 1) * 512],
                                       py[:])
                    else:
                        nc.vector.tensor_copy(
                            y_sb[:, nb * 512:(nb + 1) * 512], py[:])
                    nc.sync.dma_start(
                        y_d.ap()[tt * P:(tt + 1) * P,
                                 nb * 512:(nb + 1) * 512],
                        y_sb[:, nb * 512:(nb + 1) * 512])
                    if nb == 1:
                        del y_sbs[tt]

                def emit_work(n):
                    for _ in range(n):
                        tog[0] ^= 1
                        if work and (tog[0] or not work_ty):
                            kind, *a = work.pop(0)
                        elif work_ty:
                            kind, *a = work_ty.pop(0)
                        else:
                            return
                        if kind == "S":
                            emit_scores(*a)
                        elif kind == "XQ":
                            xq_tiles[a[0]] = load_xT(xq_d, a[0])
                        elif kind == "XV":
                            xv_tiles[a[0]] = load_xT(xv_d, a[0])
                        elif kind == "V":
                            emit_v_half(xv_tiles[a[0]], a[0], a[1],
                                        pool=typ)
                        elif kind == "Q":
                            emit_proj("q", a[0], xq_tiles[a[0]], a[1],
                                      pool=typ)
                        elif kind == "T":
                            emit_T(*a)
                        else:
                            emit_y_half(*a)

                for qb in range(4):
                    if qb == 3:
                        tail[0] = True
                    if qb < 3:
                        # next-qb Q projection + scores feed the queue;
                        # Q/XQ items spread between S items so the exp
                        # stream never pauses for a projection burst
                        ns = [("S", qb + 1, kt) for kt in range(16)]
                        if qb == 0:
                            ns2 = [("S", 2, kt) for kt in range(6)]
                            work.extend([("XQ", 2)])
                            work.extend(ns[0:4] + [("Q", 2, 0)] + ns[4:8]
                                        + [("XQ", 3)] + ns[8:12]
                                        + [("Q", 2, 1)] + ns[12:16] + ns2)
                        elif qb == 1:
                            ns = [("S", 2, kt) for kt in range(6, 16)]
                            work.extend(ns[0:4] + [("Q", 3, 0)] + ns[4:8]
                                        + [("Q", 3, 1)] + ns[8:]
                                        + [("S", 3, kt) for kt in range(4)])
                        else:
                            work.extend([("S", 3, kt) for kt in range(4, 16)])

                    for hp in range(2):
                        avq = [avp.tile([P, 4, DKA], f32, tag="avq",
                                        name=f"avq{_h}")
                               for _h in range(2)]
                        if hp == 1:
                            # late blocks run qt-major: each query tile
                            # accumulates over all 16 kt, then its norm ->
                            # transpose -> O-projection chain fires at once.
                            # This executes T/Y work inside the exp-covered
                            # span (qb2) and pipelines the kernel tail per
                            # token tile (qb3). Drain the T/Y queue first so
                            # T(qb,0) always precedes this block's Ys.
                            while work_ty:
                                knd, *a = work_ty.pop(0)
                                if knd == "T":
                                    emit_T(*a)
                                else:
                                    emit_y_half(*a)
                            a_sb = asp.tile([P, 4, 2, DK], bf16, tag="attn")
                            attn_sbs[(qb, hp)] = a_sb
                            for qt in range(4):
                                for kt in range(16):
                                    if kt % 4 == 0:
                                        emit_work(1)
                                    for hh in range(2):
                                        nc.tensor.matmul(
                                            avq[hh][:, qt, :],
                                            pts[(qb, kt, hp)][
                                                :, hh, qt * P:(qt + 1) * P],
                                            V[:, kt, 2 * hp + hh, :],
                                            start=(kt == 0), stop=(kt == 15))
                                for hh in range(2):
                                    rec2 = smp.tile([P, 1], f32, tag="rec2",
                                                    name="rec2")
                                    nc.vector.reciprocal(
                                        rec2[:], avq[hh][:, qt, DK:DKA])
                                    if tail[0] and hh == 1:
                                        nc.scalar.activation(
                                            a_sb[:, qt, hh, :],
                                            avq[hh][:, qt, 0:DK], Copy,
                                            scale=rec2[:])
                                    else:
                                        nc.vector.tensor_scalar(
                                            a_sb[:, qt, hh, :],
                                            avq[hh][:, qt, 0:DK],
                                            rec2[:], None, op0=mult)
                                emit_T(qb, hp, qts=(qt,))
                                if hp == 1:
                                    emit_y_half(4 * qb + qt, 0)
                                    emit_y_half(4 * qb + qt, 1)
                            for kt in range(16):
                                del pts[(qb, kt, hp)]
                            continue
                        for kt in range(16):
                            # PE queue is in-order: pop independent filler
                            # BEFORE the exp-gated AV matmuls so it runs
                            # during the wait. The t4=2/3 V projections are
                            # emitted here deterministically, always >= 2 kt
                            # ahead of the AV matmuls that read them.
                            if qb == 0 and hp == 0 and kt in (1, 3):
                                emit_work(1)
                            if qb == 0 and hp == 0:
                                if kt == 0:
                                    xv_tiles[2] = load_xT(xv_d, 2)
                                elif kt == 2:
                                    xv_tiles[3] = load_xT(xv_d, 3)
                                elif kt in (4, 6, 8, 10):
                                    t4v = 2 + (kt - 4) // 4
                                    emit_v_half(xv_tiles[t4v], t4v,
                                                (kt // 2) % 2, pool=typ)
                            if kt % 2 == 0:
                                emit_work(1)
                            if (hp * 16 + kt) % 4 == 0:
                                emit_work(1)
                            ptm = pts[(qb, kt, hp)]
                            for hh in range(2):
                                h = 2 * hp + hh
                                for qt in range(4):
                                    # one PSUM accumulation group per bank:
                                    # start marks the whole 2KB zero region,
                                    # each qt's first write then zero-fills
                                    nc.tensor.matmul(
                                        avq[hh][:, qt, :],
                                        ptm[:, hh, qt * P:(qt + 1) * P],
                                        V[:, kt, h, :],
                                        start=(kt == 0 and qt == 0),
                                        stop=(kt == 15 and qt == 3))
                            del pts[(qb, kt, hp)]
                        # boundary filler between the final AV and the DVE
                        # normalization chain
                        emit_work(2)
                        # partition-aligned normalization: reciprocal of the
                        # accumulated denominator column, then one
                        # tensor_scalar multiply per [128, 64] block
                        a_sb = asp.tile([P, 4, 2, DK], bf16, tag="attn")
                        attn_sbs[(qb, hp)] = a_sb
                        for hh in range(2):
                            rec = smp.tile([P, 4, 1], f32, tag="rec")
                            nc.vector.reciprocal(rec[:],
                                                 avq[hh][:, :, DK:DKA])
                            for qt in range(4):
                                # in the exp-free tail, hh1's normalize
                                # multiplies run on the idle Scalar engine
                                # (Copy activation with per-partition scale)
                                if tail[0] and hh == 1:
                                    nc.scalar.activation(
                                        a_sb[:, qt, hh, :],
                                        avq[hh][:, qt, 0:DK], Copy,
                                        scale=rec[:, qt, :])
                                else:
                                    nc.vector.tensor_scalar(
                                        a_sb[:, qt, hh, :],
                                        avq[hh][:, qt, 0:DK],
                                        rec[:, qt, :], None, op0=mult)
                        work_ty.append(("T", qb, hp))
                        if hp == 1 and qb < 3:
                                for tt in range(4 * qb, 4 * qb + 4):
                                    for nb in range(2):
                                        work_ty.append(("Y", tt, nb))
                emit_work(len(work) + len(work_ty))

                for ctx in (ty_ctx, av_ctx, sc_ctx):
                    ctx.__exit__(None, None, None)

    nc.compile()
    return nc


def _shard(query, key, value, Wq, bq, Wk, bk, Wv, bv, Wo, bo):
    import ml_dtypes
    f = np.float32
    bf = ml_dtypes.bfloat16
    q = np.asarray(query, dtype=f).astype(bf)
    k = np.asarray(key, dtype=f).astype(bf)
    v = np.asarray(value, dtype=f).astype(bf)
    qT = [np.ascontiguousarray(q[b].T) for b in range(B)]
    kT = [np.ascontiguousarray(k[b].T) for b in range(B)]
    vT = [np.ascontiguousarray(v[b].T) for b in range(B)]
    ident = np.eye(P, dtype=bf)
    in_maps = []
    for c in range(NCORES):
        b, hg = c // 4, c % 4
        ds = DS * hg
        bv_r = np.broadcast_to(
            np.asarray(bv, f)[ds:ds + DS].astype(bf)[None, :], (P, DS))
        in_maps.append({
            "xq": qT[b],
            "xk": kT[b],
            "xv": vT[b],
            "wqT": np.ascontiguousarray(np.asarray(Wq, f)[ds:ds + DS, :].T.astype(bf)),
            "wkT": np.ascontiguousarray(np.asarray(Wk, f)[ds:ds + DS, :].T.astype(bf)),
            "wvT": np.ascontiguousarray(np.asarray(Wv, f)[ds:ds + DS, :].T.astype(bf)),
            "woT": np.ascontiguousarray(np.asarray(Wo, f)[:, ds:ds + DS].T.astype(bf)),
            "bq": np.asarray(bq, f)[ds:ds + DS].reshape(2, P, 1),
            "bk": np.asarray(bk, f)[ds:ds + DS].reshape(2, P, 1),
            "bvr": np.ascontiguousarray(bv_r),
            "ident": ident,
        })
    return in_maps


def _unshard(results, bo):
    y = np.zeros((B, S, D), dtype=np.float64)
    for c in range(NCORES):
        y[c // 4] += results[c]["y"].astype(np.float64)
    y += np.asarray(bo, np.float64)
    return y.astype(np.float32)


def kernel(query, key, value, Wq, bq, Wk, bk, Wv, bv, Wo, bo):
    from concourse.bass_utils import run_bass_kernel_spmd

    if "nc" not in _cache:
        _cache["nc"] = _build()
    nc = _cache["nc"]
    in_maps = _shard(query, key, value, Wq, bq, Wk, bk, Wv, bv, Wo, bo)
    res = run_bass_kernel_spmd(nc, in_maps, core_ids=list(range(NCORES)))
    return _unshard(res.results, bo)
